# revision 25
# baseline (speedup 1.0000x reference)
"""Trainium2 Bass kernel for nn_DirectDistanceModel.

Host side (index-space layout work): per-cell last-write winner selection for
the three edge types, item_to_loc assembly, and a re-indexing of the join into
a fixed "item position" column space: pos_j = rank of item j when items are
sorted by their storage location. In that space
    item_item_dist = sum_i <S_i, L_i>
where S_i[pos_j] = seq_mat[i, j] and L_i[pos_j] = loc_mat[itl_i, itl_j] (the
loc-row value replicated over the items that share a location), both sparse
rows the host packs as (int16 position, fp16 value) winner lists.

Device side (8 NeuronCores, SPMD, sharded by item): builds the dense S and L
rows on-chip with gpsimd local_scatter (no HBM matrices, no DRAM scatter, no
AllGather), multiply-reduces them for the three scalar components, AllReduces
the scalars, and applies the 3->32->1 MLP. The start-depot row rides in block
1's unused pad partition 127 and is extracted with a one-hot mask.
"""
import numpy as np

N_ITEMS = 2000
N_STORAGE = 4094
N_LOCS = 4096
N_CORES = 8
ITEMS_PER_CORE = 250
NE = 2002          # dense row width: 2000 item positions + end slot + pad

_CACHE = {}


def _pad32(n):
    return max(32, ((int(n) + 31) // 32) * 32)


def _winners(cells, vals):
    """Last-write winner per cell (stable sort by cell, keep last)."""
    order = np.argsort(cells, kind="stable")
    cs = cells[order]
    last = np.empty(len(order), bool)
    if len(order):
        last[:-1] = cs[1:] != cs[:-1]
        last[-1] = True
    return cs[last], vals[order][last]


def _host_prep(edge_index, edge_attr, edge_type_mask):
    src = np.asarray(edge_index[0], dtype=np.int64)
    dst = np.asarray(edge_index[1], dtype=np.int64)
    mask = np.asarray(edge_type_mask, dtype=bool)
    attr = np.asarray(edge_attr, dtype=np.float32)

    ls = src - N_ITEMS
    ld = dst - N_ITEMS
    v0 = mask[:, 0] & (ls >= 0) & (ls < N_LOCS) & (ld >= 0) & (ld < N_LOCS)
    c0, val0 = _winners(ls[v0] * N_LOCS + ld[v0], attr[v0, 0])
    r0 = c0 // N_LOCS
    col0 = c0 % N_LOCS

    v1 = mask[:, 1] & (src >= 0) & (src < N_ITEMS) & (dst >= 0) & (dst < N_ITEMS)
    c1, val1 = _winners(src[v1] * N_ITEMS + dst[v1], attr[v1, 1])
    r1 = c1 // N_ITEMS
    j1 = c1 % N_ITEMS
    keep = val1 > 0.0          # reference joins only cells with seq > 0
    r1, j1, val1 = r1[keep], j1[keep], val1[keep]

    li = dst - N_ITEMS
    v2 = mask[:, 2] & (src >= 0) & (src < N_ITEMS) & (li >= 0) & (li < N_STORAGE)
    c2, w2v = _winners(src[v2], li[v2])
    itl = np.zeros(N_ITEMS, np.int64)
    itl[c2] = w2v

    # --- item position space: pos[j] = rank of item j sorted by its loc ---
    order = np.argsort(itl, kind="stable")
    pos = np.empty(N_ITEMS, np.int64)
    pos[order] = np.arange(N_ITEMS)
    itl_sorted = itl[order]
    loc_ids = np.arange(N_LOCS)
    starts = np.searchsorted(itl_sorted, loc_ids, "left")
    cnt = np.searchsorted(itl_sorted, loc_ids, "right") - starts

    # --- S: seq winners re-indexed to positions, grouped by row ---
    scnt = np.bincount(r1, minlength=N_ITEMS)
    WS = _pad32(scnt.max() if len(scnt) else 1)
    srow = np.repeat(np.arange(N_ITEMS), scnt)
    scol = np.arange(len(r1)) - np.repeat(np.cumsum(scnt) - scnt, scnt)
    S_idx = np.full((N_ITEMS, WS), -1, np.int16)
    S_val = np.zeros((N_ITEMS, WS), np.float16)
    S_idx[srow, scol] = pos[j1].astype(np.int16)
    S_val[srow, scol] = val1.astype(np.float16)

    # --- loc winners expanded to item positions, grouped by loc row ---
    rep = cnt[col0]
    exp_row = np.repeat(r0, rep)
    base = np.repeat(starts[col0], rep)
    offs = np.arange(rep.sum()) - np.repeat(np.cumsum(rep) - rep, rep)
    exp_pos = (base + offs).astype(np.int16)
    exp_val = np.repeat(val0, rep).astype(np.float16)
    rowcnt = np.bincount(exp_row, minlength=N_LOCS)
    rowstart = np.concatenate([[0], np.cumsum(rowcnt)])

    is45 = col0 == N_LOCS - 1          # winners in the end-depot column 4095
    v_end = np.zeros(N_LOCS, np.float16)
    has_end = np.zeros(N_LOCS, bool)
    v_end[r0[is45]] = val0[is45].astype(np.float16)
    has_end[r0[is45]] = True

    # --- per-item L rows: row itl[i]'s expanded list (+ end entry) ---
    ri = itl
    ilen = rowcnt[ri]
    WL = _pad32(ilen.max() + 1)
    lrow = np.repeat(np.arange(N_ITEMS), ilen)
    lcol = np.arange(ilen.sum()) - np.repeat(np.cumsum(ilen) - ilen, ilen)
    flat = np.repeat(rowstart[ri], ilen) + lcol
    L_idx = np.full((N_ITEMS, WL), -1, np.int16)
    L_val = np.zeros((N_ITEMS, WL), np.float16)
    L_idx[lrow, lcol] = exp_pos[flat]
    L_val[lrow, lcol] = exp_val[flat]
    ihas = has_end[ri]
    L_idx[ihas, ilen[ihas]] = N_ITEMS          # end slot = position 2000
    L_val[ihas, ilen[ihas]] = v_end[ri[ihas]]

    # --- start-depot row 4094 expanded; rides in block 1's pad partition 127
    n4 = int(rowcnt[N_STORAGE])
    s4 = rowstart[N_STORAGE]
    WL = max(WL, _pad32(n4))

    in_maps = []
    for c in range(N_CORES):
        lo, hi = c * ITEMS_PER_CORE, (c + 1) * ITEMS_PER_CORE
        si = np.full((256, WS), -1, np.int16)
        sv = np.zeros((256, WS), np.float16)
        si[:ITEMS_PER_CORE] = S_idx[lo:hi]
        sv[:ITEMS_PER_CORE] = S_val[lo:hi]
        lix = np.full((256, WL), -1, np.int16)
        lvx = np.zeros((256, WL), np.float16)
        lix[:ITEMS_PER_CORE] = L_idx[lo:hi]
        lvx[:ITEMS_PER_CORE] = L_val[lo:hi]
        lix[255, :n4] = exp_pos[s4:s4 + n4]
        lvx[255, :n4] = exp_val[s4:s4 + n4]
        idx = np.concatenate([si.reshape(2, 128, WS), lix.reshape(2, 128, WL)],
                             axis=2)
        val = np.concatenate([sv.reshape(2, 128, WS), lvx.reshape(2, 128, WL)],
                             axis=2)
        in_maps.append({"idx": np.ascontiguousarray(idx),
                        "val": np.ascontiguousarray(val)})
    return in_maps, WS, WL


def _build(WS, WL):
    import bass_rust as _bass_rust
    import concourse.bass as bass
    import concourse.mybir as mybir
    from concourse.library_config import all_libraries, standard
    from concourse.tile import TileContext

    F32 = mybir.dt.float32
    F16 = mybir.dt.float16
    I16 = mybir.dt.int16
    W = WS + WL

    nc = bass.Bass("TRN2")
    p = {}
    p["idx"] = nc.declare_dram_parameter("idx", [2, 128, W], I16, isOutput=False)
    p["val"] = nc.declare_dram_parameter("val", [2, 128, W], F16, isOutput=False)
    p["m127"] = nc.declare_dram_parameter("m127", [128, 1], F32, isOutput=False)
    p["W1"] = nc.declare_dram_parameter("W1", [3, 32], F32, isOutput=False)
    p["b1"] = nc.declare_dram_parameter("b1", [1, 32], F32, isOutput=False)
    p["W2"] = nc.declare_dram_parameter("W2", [32, 1], F32, isOutput=False)
    p["b2"] = nc.declare_dram_parameter("b2", [1, 1], F32, isOutput=False)
    pred = nc.declare_dram_parameter("pred", [1, 1], F32, isOutput=True)

    ar_in = nc.dram_tensor("ar_in", [1, 8], F32)
    ar_out = nc.dram_tensor("ar_out", [1, 8], F32, addr_space="Shared")

    with TileContext(nc) as tc:
        with (
            tc.tile_pool(name="p", bufs=1) as pool,
            tc.tile_pool(name="pj", bufs=2) as pj,
            tc.tile_pool(name="ps", bufs=1, space="PSUM") as psp,
        ):
            comp1 = pool.tile([128, 1], F32, tag="comp1")
            comp3 = pool.tile([128, 1], F32, tag="comp3")
            nc.vector.memset(comp1[:, :], 0.0)
            nc.vector.memset(comp3[:, :], 0.0)
            parts = pool.tile([128, 4], F32, tag="parts")
            nc.vector.memset(parts[:, :], 0.0)

            Ls = []
            for b in range(2):
                iv = pj.tile([128, W], I16, tag="iv")
                vv = pj.tile([128, W], F16, tag="vv")
                # split the two loads across the two HWDGE rings
                if b == 0:
                    nc.sync.dma_start(out=iv[:, :], in_=p["idx"][b, :, :])
                    nc.sync.dma_start(out=vv[:, :], in_=p["val"][b, :, :])
                else:
                    nc.scalar.dma_start(out=iv[:, :], in_=p["idx"][b, :, :])
                    nc.scalar.dma_start(out=vv[:, :], in_=p["val"][b, :, :])
                S = pj.tile([128, NE], F16, tag="S")
                nc.gpsimd.local_scatter(
                    out_ap=S[:, :], data_ap=vv[:, 0:WS], idxs_ap=iv[:, 0:WS],
                    channels=128, num_elems=NE, num_idxs=WS)
                L = pj.tile([128, NE], F16, tag="L")
                nc.gpsimd.local_scatter(
                    out_ap=L[:, :], data_ap=vv[:, WS:W], idxs_ap=iv[:, WS:W],
                    channels=128, num_elems=NE, num_idxs=WL)
                Ls.append(L)
                P = pj.tile([128, NE], F16, tag="P")
                nc.vector.tensor_mul(out=P[:, :], in0=S[:, :], in1=L[:, :])
                r = pj.tile([128, 1], F32, tag="r")
                nc.vector.tensor_reduce(r[:, :], P[:, 0:N_ITEMS],
                                        mybir.AxisListType.X,
                                        mybir.AluOpType.add)
                nc.vector.tensor_add(out=comp1[:, :], in0=comp1[:, :],
                                     in1=r[:, :])
                e = pj.tile([128, 1], F32, tag="e")
                nc.vector.tensor_copy(out=e[:, :],
                                      in_=L[:, N_ITEMS:N_ITEMS + 1])
                nc.vector.tensor_add(out=comp3[:, :], in0=comp3[:, :],
                                     in1=e[:, :])

            # start depot: row 4094 rode in as block 1 partition 127's L row;
            # reduce every partition's L row and mask to partition 127
            m127 = pool.tile([128, 1], F32, tag="m127")
            nc.sync.dma_start(out=m127[:, :], in_=p["m127"][:, :])
            rL = pool.tile([128, 1], F32, tag="rL")
            nc.vector.tensor_reduce(rL[:, :], Ls[1][:, 0:N_ITEMS],
                                    mybir.AxisListType.X, mybir.AluOpType.add)
            nc.vector.tensor_mul(out=parts[:, 1:2], in0=rL[:, :],
                                 in1=m127[:, :])

            nc.vector.tensor_copy(out=parts[:, 0:1], in_=comp1[:, :])
            nc.vector.tensor_copy(out=parts[:, 2:3], in_=comp3[:, :])

            # ---------- partition-reduce via matmul ----------
            ones = pool.tile([128, 1], F32, tag="ones")
            nc.vector.memset(ones[:, :], 1.0)
            psum3 = psp.tile([1, 4], F32, tag="psum3")
            nc.tensor.matmul(psum3[:, :], ones[:, :], parts[:, :],
                             start=True, stop=True)
            packed = pool.tile([1, 8], F32, tag="packed")
            nc.vector.memset(packed[:, :], 0.0)
            nc.vector.tensor_copy(out=packed[0:1, 0:1], in_=psum3[0:1, 0:1])
            nc.vector.tensor_scalar(out=packed[0:1, 1:2],
                                    in0=psum3[0:1, 1:2], scalar1=0.125,
                                    scalar2=None, op0=mybir.AluOpType.mult)
            nc.vector.tensor_copy(out=packed[0:1, 2:3], in_=psum3[0:1, 2:3])
            nc.sync.dma_start(out=ar_in[:, :], in_=packed[:, :])
            nc.gpsimd.collective_compute(
                "AllReduce",
                mybir.AluOpType.add,
                replica_groups=[list(range(N_CORES))],
                ins=[ar_in[:, :]],
                outs=[ar_out[:, :]],
            )

            # ---------- MLP ----------
            comps3 = pool.tile([3, 1], F32, tag="comps3")
            nc.sync.dma_start(out=comps3[:, :],
                              in_=ar_out[0:1, 0:3].rearrange("one k -> k one"))
            w1 = pool.tile([3, 32], F32, tag="w1")
            nc.sync.dma_start(out=w1[:, :], in_=p["W1"][:, :])
            b1 = pool.tile([1, 32], F32, tag="b1")
            nc.sync.dma_start(out=b1[:, :], in_=p["b1"][:, :])
            hpsum = psp.tile([1, 32], F32, tag="hpsum")
            nc.tensor.matmul(hpsum[:, :], comps3[:, :], w1[:, :],
                             start=True, stop=True)
            h = pool.tile([1, 32], F32, tag="h")
            nc.vector.tensor_add(out=h[:, :], in0=hpsum[:, :], in1=b1[:, :])
            hr = pool.tile([1, 32], F32, tag="hr")
            nc.vector.tensor_relu(out=hr[:, :], in_=h[:, :])
            w2 = pool.tile([1, 32], F32, tag="w2")
            nc.sync.dma_start(out=w2[:, :],
                              in_=p["W2"][:, :].rearrange("k one -> one k"))
            hw = pool.tile([1, 32], F32, tag="hw")
            nc.vector.tensor_mul(out=hw[:, :], in0=hr[:, :], in1=w2[:, :])
            out1 = pool.tile([1, 1], F32, tag="out1")
            nc.vector.tensor_reduce(out1[:, :], hw[:, :], mybir.AxisListType.X,
                                    mybir.AluOpType.add)
            b2 = pool.tile([1, 1], F32, tag="b2t")
            nc.sync.dma_start(out=b2[:, :], in_=p["b2"][:, :])
            nc.vector.tensor_add(out=out1[:, :], in0=out1[:, :], in1=b2[:, :])
            nc.sync.dma_start(out=pred[:, :], in_=out1[:, :])

    inst_type_to_lib_mask = {}
    for lib in all_libraries:
        for t in lib.instructions:
            inst_type_to_lib_mask[t] = (
                inst_type_to_lib_mask.get(t, 0) | (1 << lib.index))
    _bass_rust.insert_library_loads(nc, inst_type_to_lib_mask,
                                    len(all_libraries), standard.index)
    mybir.codegen_inst_isa_subclasses(nc)
    _split_sync_waits(nc)
    return nc


def _split_sync_waits(nc, max_waits=1):
    import concourse.mybir as mybir
    ctr = [0]
    for f in nc.m.functions:
        for bb in f.blocks:
            new_insts = []
            for inst in bb.instructions:
                si = getattr(inst, "sync_info", None)
                if si is not None and si.on_wait and len(si.on_wait) > max_waits:
                    waits = list(si.on_wait)
                    head, tail = waits[:-max_waits], waits[-max_waits:]
                    while head:
                        chunk, head = head[:max_waits], head[max_waits:]
                        ctr[0] += 1
                        nop = mybir.InstNoOp(
                            name=f"I-syncfix-{ctr[0]}",
                            engine=inst.engine,
                            ins=[],
                            outs=[],
                            sync_info=mybir.SyncInfo(on_wait=chunk,
                                                     on_update=[]),
                            bass_nofuse=True,
                        )
                        new_insts.append(nop)
                    inst.sync_info = mybir.SyncInfo(
                        on_wait=tail, on_update=list(si.on_update))
                new_insts.append(inst)
            bb.instructions[:] = new_insts


def kernel(**inputs):
    import os
    from concourse.bass_utils import run_bass_kernel_spmd

    edge_index = np.asarray(inputs["edge_index"])
    edge_attr = np.asarray(inputs["edge_attr"])
    edge_type_mask = np.asarray(inputs["edge_type_mask"])
    assert int(inputs["n_items"]) == N_ITEMS
    assert int(inputs["n_storage"]) == N_STORAGE
    assert int(inputs["n_locs"]) == N_LOCS

    in_maps, WS, WL = _host_prep(edge_index, edge_attr, edge_type_mask)
    W1 = np.asarray(inputs["W1"], np.float32).reshape(3, 32)
    b1 = np.asarray(inputs["b1"], np.float32).reshape(1, 32)
    W2 = np.asarray(inputs["W2"], np.float32).reshape(32, 1)
    b2 = np.asarray(inputs["b2"], np.float32).reshape(1, 1)
    m127 = np.zeros((128, 1), np.float32)
    m127[127, 0] = 1.0
    for m in in_maps:
        m["W1"] = W1
        m["b1"] = b1
        m["W2"] = W2
        m["b2"] = b2
        m["m127"] = m127

    key = (WS, WL)
    if key not in _CACHE:
        _CACHE[key] = _build(*key)
    nc = _CACHE[key]
    trace = os.environ.get("KERNEL_TRACE") == "1"
    res = run_bass_kernel_spmd(nc, in_maps, core_ids=list(range(N_CORES)),
                               trace=trace)
    if trace and res.exec_time_ns is not None:
        print(f"HW exec time: {res.exec_time_ns} ns")
    out = res.results[0]["pred"]
    return np.float32(out.reshape(())).astype(np.float32)


# revision 26
# speedup vs baseline: 1.1114x; 1.1114x over previous
"""Trainium2 Bass kernel for nn_DirectDistanceModel.

Host side (index-space layout work): per-cell last-write winner selection for
the three edge types, item_to_loc assembly, and a re-indexing of the join into
a fixed "item position" column space: pos_j = rank of item j when items are
sorted by their storage location. In that space
    item_item_dist = sum_i <S_i, L_i>
where S_i[pos_j] = seq_mat[i, j] and L_i[pos_j] = loc_mat[itl_i, itl_j] (the
loc-row value replicated over the items that share a location), both sparse
rows the host packs as (int16 position, fp16 value) winner lists.

Device side (8 NeuronCores, SPMD, sharded by item): builds the dense S and L
rows on-chip with gpsimd local_scatter (no HBM matrices, no DRAM scatter, no
AllGather), multiply-reduces them for the three scalar components, AllReduces
the scalars, and applies the 3->32->1 MLP. The start-depot row rides in block
1's unused pad partition 127 and is extracted with a one-hot mask.
"""
import numpy as np

N_ITEMS = 2000
N_STORAGE = 4094
N_LOCS = 4096
N_CORES = 8
ITEMS_PER_CORE = 250
NE = 2002          # dense row width: 2000 item positions + end slot + pad

_CACHE = {}


def _pad32(n):
    return max(32, ((int(n) + 31) // 32) * 32)


def _winners(cells, vals):
    """Last-write winner per cell (stable sort by cell, keep last)."""
    order = np.argsort(cells, kind="stable")
    cs = cells[order]
    last = np.empty(len(order), bool)
    if len(order):
        last[:-1] = cs[1:] != cs[:-1]
        last[-1] = True
    return cs[last], vals[order][last]


def _host_prep(edge_index, edge_attr, edge_type_mask):
    src = np.asarray(edge_index[0], dtype=np.int64)
    dst = np.asarray(edge_index[1], dtype=np.int64)
    mask = np.asarray(edge_type_mask, dtype=bool)
    attr = np.asarray(edge_attr, dtype=np.float32)

    ls = src - N_ITEMS
    ld = dst - N_ITEMS
    v0 = mask[:, 0] & (ls >= 0) & (ls < N_LOCS) & (ld >= 0) & (ld < N_LOCS)
    c0, val0 = _winners(ls[v0] * N_LOCS + ld[v0], attr[v0, 0])
    r0 = c0 // N_LOCS
    col0 = c0 % N_LOCS

    v1 = mask[:, 1] & (src >= 0) & (src < N_ITEMS) & (dst >= 0) & (dst < N_ITEMS)
    c1, val1 = _winners(src[v1] * N_ITEMS + dst[v1], attr[v1, 1])
    r1 = c1 // N_ITEMS
    j1 = c1 % N_ITEMS
    keep = val1 > 0.0          # reference joins only cells with seq > 0
    r1, j1, val1 = r1[keep], j1[keep], val1[keep]

    li = dst - N_ITEMS
    v2 = mask[:, 2] & (src >= 0) & (src < N_ITEMS) & (li >= 0) & (li < N_STORAGE)
    c2, w2v = _winners(src[v2], li[v2])
    itl = np.zeros(N_ITEMS, np.int64)
    itl[c2] = w2v

    # --- item position space: pos[j] = rank of item j sorted by its loc ---
    order = np.argsort(itl, kind="stable")
    pos = np.empty(N_ITEMS, np.int64)
    pos[order] = np.arange(N_ITEMS)
    itl_sorted = itl[order]
    loc_ids = np.arange(N_LOCS)
    starts = np.searchsorted(itl_sorted, loc_ids, "left")
    cnt = np.searchsorted(itl_sorted, loc_ids, "right") - starts

    # --- S: seq winners re-indexed to positions, grouped by row ---
    scnt = np.bincount(r1, minlength=N_ITEMS)
    WS = _pad32(scnt.max() if len(scnt) else 1)
    srow = np.repeat(np.arange(N_ITEMS), scnt)
    scol = np.arange(len(r1)) - np.repeat(np.cumsum(scnt) - scnt, scnt)
    S_idx = np.full((N_ITEMS, WS), -1, np.int16)
    S_val = np.zeros((N_ITEMS, WS), np.float16)
    S_idx[srow, scol] = pos[j1].astype(np.int16)
    S_val[srow, scol] = val1.astype(np.float16)

    # --- loc winners expanded to item positions, grouped by loc row ---
    rep = cnt[col0]
    exp_row = np.repeat(r0, rep)
    base = np.repeat(starts[col0], rep)
    offs = np.arange(rep.sum()) - np.repeat(np.cumsum(rep) - rep, rep)
    exp_pos = (base + offs).astype(np.int16)
    exp_val = np.repeat(val0, rep).astype(np.float16)
    rowcnt = np.bincount(exp_row, minlength=N_LOCS)
    rowstart = np.concatenate([[0], np.cumsum(rowcnt)])

    is45 = col0 == N_LOCS - 1          # winners in the end-depot column 4095
    v_end = np.zeros(N_LOCS, np.float16)
    has_end = np.zeros(N_LOCS, bool)
    v_end[r0[is45]] = val0[is45].astype(np.float16)
    has_end[r0[is45]] = True

    # --- per-item L rows: row itl[i]'s expanded list (+ end entry) ---
    ri = itl
    ilen = rowcnt[ri]
    WL = _pad32(ilen.max() + 1)
    lrow = np.repeat(np.arange(N_ITEMS), ilen)
    lcol = np.arange(ilen.sum()) - np.repeat(np.cumsum(ilen) - ilen, ilen)
    flat = np.repeat(rowstart[ri], ilen) + lcol
    L_idx = np.full((N_ITEMS, WL), -1, np.int16)
    L_val = np.zeros((N_ITEMS, WL), np.float16)
    L_idx[lrow, lcol] = exp_pos[flat]
    L_val[lrow, lcol] = exp_val[flat]
    ihas = has_end[ri]
    L_idx[ihas, ilen[ihas]] = N_ITEMS          # end slot = position 2000
    L_val[ihas, ilen[ihas]] = v_end[ri[ihas]]

    # --- start-depot row 4094 expanded; rides in block 1's pad partition 127
    n4 = int(rowcnt[N_STORAGE])
    s4 = rowstart[N_STORAGE]
    WL = max(WL, _pad32(n4))

    in_maps = []
    for c in range(N_CORES):
        lo, hi = c * ITEMS_PER_CORE, (c + 1) * ITEMS_PER_CORE
        si = np.full((256, WS), -1, np.int16)
        sv = np.zeros((256, WS), np.float16)
        si[:ITEMS_PER_CORE] = S_idx[lo:hi]
        sv[:ITEMS_PER_CORE] = S_val[lo:hi]
        lix = np.full((256, WL), -1, np.int16)
        lvx = np.zeros((256, WL), np.float16)
        lix[:ITEMS_PER_CORE] = L_idx[lo:hi]
        lvx[:ITEMS_PER_CORE] = L_val[lo:hi]
        lix[255, :n4] = exp_pos[s4:s4 + n4]
        lvx[255, :n4] = exp_val[s4:s4 + n4]
        idx = np.concatenate([si.reshape(2, 128, WS), lix.reshape(2, 128, WL)],
                             axis=2)
        val = np.concatenate([sv.reshape(2, 128, WS), lvx.reshape(2, 128, WL)],
                             axis=2)
        in_maps.append({"idx": np.ascontiguousarray(idx),
                        "val": np.ascontiguousarray(val)})
    return in_maps, WS, WL


def _build(WS, WL):
    import bass_rust as _bass_rust
    import concourse.bass as bass
    import concourse.mybir as mybir
    from concourse.library_config import all_libraries, standard
    from concourse.tile import TileContext

    F32 = mybir.dt.float32
    F16 = mybir.dt.float16
    I16 = mybir.dt.int16
    W = WS + WL

    nc = bass.Bass("TRN2")
    p = {}
    p["idx"] = nc.declare_dram_parameter("idx", [2, 128, W], I16, isOutput=False)
    p["val"] = nc.declare_dram_parameter("val", [2, 128, W], F16, isOutput=False)
    p["m127"] = nc.declare_dram_parameter("m127", [128, 1], F32, isOutput=False)
    p["W1"] = nc.declare_dram_parameter("W1", [3, 32], F32, isOutput=False)
    p["b1"] = nc.declare_dram_parameter("b1", [1, 32], F32, isOutput=False)
    p["W2"] = nc.declare_dram_parameter("W2", [32, 1], F32, isOutput=False)
    p["b2"] = nc.declare_dram_parameter("b2", [1, 1], F32, isOutput=False)
    pred = nc.declare_dram_parameter("pred", [1, 1], F32, isOutput=True)

    ar_in = nc.dram_tensor("ar_in", [1, 8], F32)
    ar_out = nc.dram_tensor("ar_out", [1, 8], F32, addr_space="Shared")

    with TileContext(nc) as tc:
        with (
            tc.tile_pool(name="p", bufs=1) as pool,
            tc.tile_pool(name="pj", bufs=2) as pj,
            tc.tile_pool(name="ps", bufs=1, space="PSUM") as psp,
        ):
            parts = pool.tile([128, 3], F32, tag="parts")

            rs, Ls, vvs = [], [], []
            for b in range(2):
                iv = pj.tile([128, W], I16, tag="iv")
                vv = pj.tile([128, W], F16, tag="vv")
                # split the two loads across the two HWDGE rings
                if b == 0:
                    nc.sync.dma_start(out=iv[:, :], in_=p["idx"][b, :, :])
                    nc.sync.dma_start(out=vv[:, :], in_=p["val"][b, :, :])
                else:
                    nc.scalar.dma_start(out=iv[:, :], in_=p["idx"][b, :, :])
                    nc.scalar.dma_start(out=vv[:, :], in_=p["val"][b, :, :])
                S = pj.tile([128, NE], F16, tag="S")
                nc.gpsimd.local_scatter(
                    out_ap=S[:, :], data_ap=vv[:, 0:WS], idxs_ap=iv[:, 0:WS],
                    channels=128, num_elems=NE, num_idxs=WS)
                L = pj.tile([128, NE], F16, tag="L")
                nc.gpsimd.local_scatter(
                    out_ap=L[:, :], data_ap=vv[:, WS:W], idxs_ap=iv[:, WS:W],
                    channels=128, num_elems=NE, num_idxs=WL)
                P = pj.tile([128, NE], F16, tag="P")
                nc.vector.tensor_mul(out=P[:, :], in0=S[:, :], in1=L[:, :])
                r = pj.tile([128, 1], F32, tag="r")
                nc.vector.tensor_reduce(r[:, :], P[:, 0:N_ITEMS],
                                        mybir.AxisListType.X,
                                        mybir.AluOpType.add)
                rs.append(r)
                Ls.append(L)
                vvs.append(vv)

            # start depot: row 4094 rode in as block 1 partition 127's L
            # winner list; a scatter permutes, so its sum equals the sum of
            # the raw packed values — no need to wait for the dense row
            m127 = pool.tile([128, 1], F32, tag="m127")
            nc.sync.dma_start(out=m127[:, :], in_=p["m127"][:, :])
            rL = pool.tile([128, 1], F32, tag="rL")
            nc.vector.tensor_reduce(rL[:, :], vvs[1][:, WS:W],
                                    mybir.AxisListType.X, mybir.AluOpType.add)
            nc.vector.tensor_mul(out=parts[:, 1:2], in0=rL[:, :],
                                 in1=m127[:, :])

            nc.vector.tensor_add(out=parts[:, 0:1], in0=rs[0][:, :],
                                 in1=rs[1][:, :])
            nc.vector.tensor_add(out=parts[:, 2:3],
                                 in0=Ls[0][:, N_ITEMS:N_ITEMS + 1],
                                 in1=Ls[1][:, N_ITEMS:N_ITEMS + 1])

            # ---------- partition-reduce via matmul ----------
            ones = pool.tile([128, 1], F32, tag="ones")
            nc.vector.memset(ones[:, :], 1.0)
            psum3 = psp.tile([1, 3], F32, tag="psum3")
            nc.tensor.matmul(psum3[:, :], ones[:, :], parts[:, :],
                             start=True, stop=True)
            packed = pool.tile([1, 8], F32, tag="packed")
            nc.vector.memset(packed[:, :], 0.0)
            nc.vector.tensor_copy(out=packed[0:1, 0:1], in_=psum3[0:1, 0:1])
            nc.vector.tensor_scalar(out=packed[0:1, 1:2],
                                    in0=psum3[0:1, 1:2], scalar1=0.125,
                                    scalar2=None, op0=mybir.AluOpType.mult)
            nc.vector.tensor_copy(out=packed[0:1, 2:3], in_=psum3[0:1, 2:3])
            nc.sync.dma_start(out=ar_in[:, :], in_=packed[:, :])
            nc.gpsimd.collective_compute(
                "AllReduce",
                mybir.AluOpType.add,
                replica_groups=[list(range(N_CORES))],
                ins=[ar_in[:, :]],
                outs=[ar_out[:, :]],
            )

            # ---------- MLP ----------
            comps3 = pool.tile([3, 1], F32, tag="comps3")
            nc.sync.dma_start(out=comps3[:, :],
                              in_=ar_out[0:1, 0:3].rearrange("one k -> k one"))
            w1 = pool.tile([3, 32], F32, tag="w1")
            nc.sync.dma_start(out=w1[:, :], in_=p["W1"][:, :])
            b1 = pool.tile([1, 32], F32, tag="b1")
            nc.sync.dma_start(out=b1[:, :], in_=p["b1"][:, :])
            hpsum = psp.tile([1, 32], F32, tag="hpsum")
            nc.tensor.matmul(hpsum[:, :], comps3[:, :], w1[:, :],
                             start=True, stop=True)
            h = pool.tile([1, 32], F32, tag="h")
            nc.vector.tensor_add(out=h[:, :], in0=hpsum[:, :], in1=b1[:, :])
            hr = pool.tile([1, 32], F32, tag="hr")
            nc.vector.tensor_relu(out=hr[:, :], in_=h[:, :])
            w2 = pool.tile([1, 32], F32, tag="w2")
            nc.sync.dma_start(out=w2[:, :],
                              in_=p["W2"][:, :].rearrange("k one -> one k"))
            hw = pool.tile([1, 32], F32, tag="hw")
            nc.vector.tensor_mul(out=hw[:, :], in0=hr[:, :], in1=w2[:, :])
            out1 = pool.tile([1, 1], F32, tag="out1")
            nc.vector.tensor_reduce(out1[:, :], hw[:, :], mybir.AxisListType.X,
                                    mybir.AluOpType.add)
            b2 = pool.tile([1, 1], F32, tag="b2t")
            nc.sync.dma_start(out=b2[:, :], in_=p["b2"][:, :])
            nc.vector.tensor_add(out=out1[:, :], in0=out1[:, :], in1=b2[:, :])
            nc.sync.dma_start(out=pred[:, :], in_=out1[:, :])

    inst_type_to_lib_mask = {}
    for lib in all_libraries:
        for t in lib.instructions:
            inst_type_to_lib_mask[t] = (
                inst_type_to_lib_mask.get(t, 0) | (1 << lib.index))
    _bass_rust.insert_library_loads(nc, inst_type_to_lib_mask,
                                    len(all_libraries), standard.index)
    mybir.codegen_inst_isa_subclasses(nc)
    _split_sync_waits(nc)
    return nc


def _split_sync_waits(nc, max_waits=1):
    import concourse.mybir as mybir
    ctr = [0]
    for f in nc.m.functions:
        for bb in f.blocks:
            new_insts = []
            for inst in bb.instructions:
                si = getattr(inst, "sync_info", None)
                if si is not None and si.on_wait and len(si.on_wait) > max_waits:
                    waits = list(si.on_wait)
                    head, tail = waits[:-max_waits], waits[-max_waits:]
                    while head:
                        chunk, head = head[:max_waits], head[max_waits:]
                        ctr[0] += 1
                        nop = mybir.InstNoOp(
                            name=f"I-syncfix-{ctr[0]}",
                            engine=inst.engine,
                            ins=[],
                            outs=[],
                            sync_info=mybir.SyncInfo(on_wait=chunk,
                                                     on_update=[]),
                            bass_nofuse=True,
                        )
                        new_insts.append(nop)
                    inst.sync_info = mybir.SyncInfo(
                        on_wait=tail, on_update=list(si.on_update))
                new_insts.append(inst)
            bb.instructions[:] = new_insts


def kernel(**inputs):
    import os
    from concourse.bass_utils import run_bass_kernel_spmd

    edge_index = np.asarray(inputs["edge_index"])
    edge_attr = np.asarray(inputs["edge_attr"])
    edge_type_mask = np.asarray(inputs["edge_type_mask"])
    assert int(inputs["n_items"]) == N_ITEMS
    assert int(inputs["n_storage"]) == N_STORAGE
    assert int(inputs["n_locs"]) == N_LOCS

    in_maps, WS, WL = _host_prep(edge_index, edge_attr, edge_type_mask)
    W1 = np.asarray(inputs["W1"], np.float32).reshape(3, 32)
    b1 = np.asarray(inputs["b1"], np.float32).reshape(1, 32)
    W2 = np.asarray(inputs["W2"], np.float32).reshape(32, 1)
    b2 = np.asarray(inputs["b2"], np.float32).reshape(1, 1)
    m127 = np.zeros((128, 1), np.float32)
    m127[127, 0] = 1.0
    for m in in_maps:
        m["W1"] = W1
        m["b1"] = b1
        m["W2"] = W2
        m["b2"] = b2
        m["m127"] = m127

    key = (WS, WL)
    if key not in _CACHE:
        _CACHE[key] = _build(*key)
    nc = _CACHE[key]
    trace = os.environ.get("KERNEL_TRACE") == "1"
    res = run_bass_kernel_spmd(nc, in_maps, core_ids=list(range(N_CORES)),
                               trace=trace)
    if trace and res.exec_time_ns is not None:
        print(f"HW exec time: {res.exec_time_ns} ns")
    out = res.results[0]["pred"]
    return np.float32(out.reshape(())).astype(np.float32)
